# revision 27
# baseline (speedup 1.0000x reference)
"""Trainium2 Bass kernel for nn_CGBlock (gnn_message_passing).

Computation (B=256, S=512, D=128):
  c_out = c_mlp(c)                  # [B,D] MLP with BatchNorm over batch
  s_h   = s_mlp(s)                  # [B,S,D] MLP, BN stats over batch axis 0
                                    #   (independent per (seq, feature))
  s_out = s_h * c_out[:, None, :]
  agg   = max(s_out, axis=1)
  returns (s_out, agg)

Sharding: the s-MLP BatchNorm is independent per sequence position, so we
shard the SEQUENCE axis across the 8 cores (64 positions each). Every core
holds the full batch for its slice, so BN needs no cross-device collectives.
The tiny c-MLP is computed redundantly on every core.

On-device layout is feature-major ([D, seq, batch]) so the PE array can
contract over D directly; the host transposes in/out (layout prep only).

BN+ReLU is fused into one ACT instruction per sequence position:
  t = relu(kk*h + nkb),  kk = g/std,  nkb = be - mean*kk
(biases b1/b2 cancel inside BN; relu commutes with kk > 0). The stats
combine is shortened by folding per-feature scales into the ACT sqrt:
  1/kk = sqrt( (256*var) * 1/(256 g^2) + eps/g^2 )
with 1/(256 g^2) and eps/g^2 precomputed on the host.
"""
import sys
import os

for _p in ('/opt/trn_rl_repo', '/root/.axon_site/_ro/trn_rl_repo'):
    if os.path.isdir(_p) and _p not in sys.path:
        sys.path.insert(0, _p)

import numpy as np
from contextlib import ExitStack

import concourse.bass as bass
import concourse.bacc as bacc
import concourse.tile as tile
import concourse.mybir as mybir
from concourse.bass_utils import run_bass_kernel_spmd

f32 = mybir.dt.float32
f32r = mybir.dt.float32r
AF = mybir.ActivationFunctionType
ALU = mybir.AluOpType

B, S, D = 256, 512, 128
NCORES = 8
S_LOC = S // NCORES          # 64 sequence positions per core
N_LOC = S_LOC * B            # 16384 columns per core
GRP_S = 8                    # sequence positions per pipeline group
NGRP = S_LOC // GRP_S        # 8 groups
GCOLS = GRP_S * B            # 2048 columns per group
NCHUNK = GCOLS // 512        # 512-col psum chunks per group (4)
EPS = 1e-5

GATE_DVE_MOD = (0, 1, 2)     # 512-col chunks (mod 4) gated on DVE direct path

LAST_RESULTS = None
_CACHE = {}

# vecs columns (packed per-partition vectors, host-precomputed):
# 0: s1_scale = 1/(256*g1^2)    1: s1_bias = eps/g1^2   2: s_be1
# 3: s2_scale                   4: s2_bias              5: s_be2
# 6: s_b3
# 7: c1_scale                   8: c1_bias              9: c_be1
# 10: c2_scale                 11: c2_bias             12: c_be2
# 13: c_b3   14: 64.0   15: -0.5
NVEC = 16


def _build_nc(with_be):
    nc = bacc.Bacc("TRN2", target_bir_lowering=False, debug=False)

    xT = nc.dram_tensor("xT", [D, N_LOC], f32r, kind="ExternalInput").ap()
    # packed consts: 6 weights [D,D] + cT [D,B] + vecs [D,NVEC]
    CPACK = 6 * D + B + NVEC
    cpack = nc.dram_tensor("cpack", [D, CPACK], f32r,
                           kind="ExternalInput").ap()

    outT = nc.dram_tensor("outT", [D, N_LOC], f32, kind="ExternalOutput").ap()

    with tile.TileContext(nc) as tc, ExitStack() as ctx:
        cpool = ctx.enter_context(tc.tile_pool(name="consts", bufs=1))
        xpool = ctx.enter_context(tc.tile_pool(name="xin", bufs=4))
        t1pool = ctx.enter_context(tc.tile_pool(name="t1", bufs=3))
        t2pool = ctx.enter_context(tc.tile_pool(name="t2", bufs=3))
        sopool = ctx.enter_context(tc.tile_pool(name="sout", bufs=3))
        stpool = ctx.enter_context(tc.tile_pool(name="stats", bufs=4))
        smpool = ctx.enter_context(tc.tile_pool(name="smalls", bufs=4))
        cmpool = ctx.enter_context(tc.tile_pool(name="cmlp", bufs=2))
        pspool = ctx.enter_context(tc.tile_pool(name="ps", bufs=8, space="PSUM"))

        # ---- load constants (single DMA) ----
        wnames = ["w_s1", "w_s2", "w_s3", "w_c1", "w_c2", "w_c3"]
        cpk = cpool.tile([D, CPACK], f32r, tag="cpack")
        nc.sync.dma_start(cpk[:], cpack)
        wts = {nm: cpk[:, i * D:(i + 1) * D] for i, nm in enumerate(wnames)}
        ct = cpk[:, 6 * D:6 * D + B]
        vt = cpk[:, 6 * D + B:].bitcast(f32)

        def vc(i):
            return vt[:, i:i + 1]

        s_b3 = vc(6)

        # ---- stats combine: even/odd bn_stats -> (kk, nkb) [128, n] ----
        # kk = g/std ; nkb = be - mean*kk
        def combine_stats(st, n, sc_ap, sb_ap, be_ap, tag):
            me = st[:, :, 1]; mo = st[:, :, 4]
            cve = st[:, :, 2]; cvo = st[:, :, 5]
            Ssum = smpool.tile([D, n], f32, tag=f"S_{tag}")
            nc.gpsimd.tensor_tensor(Ssum[:], me, mo, op=ALU.add)
            Dd = smpool.tile([D, n], f32, tag=f"D_{tag}")
            nc.gpsimd.tensor_tensor(Dd[:], me, mo, op=ALU.subtract)
            D2 = smpool.tile([D, n], f32, tag=f"D2_{tag}")
            nc.gpsimd.tensor_tensor(D2[:], Dd[:], Dd[:], op=ALU.mult)
            Q = smpool.tile([D, n], f32, tag=f"Q_{tag}")
            nc.gpsimd.tensor_tensor(Q[:], cve, cvo, op=ALU.add)
            # w = 256*var = D2*64 + Q
            w = smpool.tile([D, n], f32, tag=f"w_{tag}")
            nc.vector.scalar_tensor_tensor(w[:], D2[:], 64.0, Q[:],
                                           op0=ALU.mult, op1=ALU.add)
            # kk = g/std = rsqrt(w*sc + sb)  (one ACT op; arg is positive)
            kk = smpool.tile([D, n], f32, tag=f"kk_{tag}")
            nc.scalar.activation(kk[:], w[:], AF.Abs_reciprocal_sqrt,
                                 bias=sb_ap, scale=sc_ap)
            nkb = smpool.tile([D, n], f32, tag=f"nkb_{tag}")
            nc.vector.scalar_tensor_tensor(nkb[:], Ssum[:], -0.5, kk[:],
                                           op0=ALU.mult, op1=ALU.mult)
            if with_be:
                nkb2 = smpool.tile([D, n], f32, tag=f"nkb2_{tag}")
                nc.vector.tensor_scalar(nkb2[:], nkb[:], be_ap, None,
                                        op0=ALU.add)
                nkb = nkb2
            return kk, nkb

        # ---- s-MLP: software-pipelined over groups ----
        x_tiles = [None] * NGRP

        def dma_in(g):
            xg = xpool.tile([D, GCOLS], f32r, tag="xg")
            nc.gpsimd.dma_start(xg[:], xT[:, g * GCOLS:(g + 1) * GCOLS])
            x_tiles[g] = xg

        def mm_layer(wname, src_tile, sttag):
            """4 matmuls [D,512] + 8 per-s bn_stats."""
            ps = []
            st = stpool.tile([D, GRP_S, 6], f32, tag=sttag)
            for cchunk in range(NCHUNK):
                pt = pspool.tile([D, 512], f32, tag="pt")
                nc.tensor.matmul(pt[:], wts[wname],
                                 src_tile[:, cchunk * 512:(cchunk + 1) * 512],
                                 start=True, stop=True)
                for half in range(2):
                    si = cchunk * 2 + half
                    nc.vector.bn_stats(st[:, si, :],
                                       pt[:, half * B:(half + 1) * B])
                ps.append(pt)
            return ps, st

        def norm_layer(ps, kk, nkb, dpool, dtag):
            dst_t = dpool.tile([D, GCOLS], f32r, tag=dtag)
            for si in range(GRP_S):
                pt = ps[si // 2]
                src = pt[:, (si % 2) * B:(si % 2 + 1) * B]
                nc.scalar.activation(dst_t[:, si * B:(si + 1) * B], src,
                                     AF.Relu, bias=nkb[:, si:si + 1],
                                     scale=kk[:, si:si + 1])
            return dst_t

        def l3_gate_out(t2, g):
            so = sopool.tile([D, GCOLS], f32, tag="so")
            for spair in range(NCHUNK):
                pt = pspool.tile([D, 512], f32, tag="pt")
                nc.tensor.matmul(pt[:], wts["w_s3"],
                                 t2[:, spair * 512:(spair + 1) * 512],
                                 start=True, stop=True)
                dstv = so[:, spair * 512:(spair + 1) * 512]
                if spair % 4 in GATE_DVE_MOD:
                    nc.vector.scalar_tensor_tensor(
                        dstv.rearrange("p (s b) -> p s b", b=B),
                        pt[:].rearrange("p (s b) -> p s b", b=B),
                        s_b3,
                        c_out[:].unsqueeze(1).broadcast_to([D, 2, B]),
                        op0=ALU.add, op1=ALU.mult)
                else:
                    u = smpool.tile([D, 512], f32, tag="u")
                    nc.scalar.activation(u[:], pt[:], AF.Identity,
                                         bias=s_b3, scale=1.0)
                    nc.gpsimd.tensor_tensor(
                        dstv.rearrange("p (s b) -> p s b", b=B),
                        u[:].rearrange("p (s b) -> p s b", b=B),
                        c_out[:].unsqueeze(1).broadcast_to([D, 2, B]),
                        op=ALU.mult)
                nc.sync.dma_start(
                    outT[:, g * GCOLS + spair * 512:
                         g * GCOLS + (spair + 1) * 512], dstv)

        # prologue
        dma_in(0)
        dma_in(1)
        dma_in(2)
        st_l1 = {}
        st_l2 = {}
        ps1, st1 = mm_layer("w_s1", x_tiles[0], "st1")
        kn1 = combine_stats(st1, GRP_S, vc(0), vc(1), vc(2), "l1")
        st_l1[0] = (ps1, kn1)

        # ---- c-MLP (tiny, replicated on every core) ----
        cc = ct
        for li, (wn, sc_i, sb_i, be_i) in enumerate(
                [("w_c1", 7, 8, 9), ("w_c2", 10, 11, 12)]):
            pcf = pspool.tile([D, 512], f32, tag="pt")
            pc = pcf[:, 0:B]
            nc.tensor.matmul(pc, wts[wn], cc[:], start=True, stop=True)
            st = stpool.tile([D, 1, 6], f32, tag=f"stc{li}")
            nc.vector.bn_stats(st[:, 0, :], pc)
            kk, nkb = combine_stats(st, 1, vc(sc_i), vc(sb_i), vc(be_i),
                                    f"c{li}")
            nxt = cmpool.tile([D, B], f32r, tag=f"cact{li}")
            nc.scalar.activation(nxt[:], pc, AF.Relu,
                                 bias=nkb[:, 0:1], scale=kk[:, 0:1])
            cc = nxt
        pcf = pspool.tile([D, 512], f32, tag="pt")
        pc = pcf[:, 0:B]
        nc.tensor.matmul(pc, wts["w_c3"], cc[:], start=True, stop=True)
        c_out = cpool.tile([D, B], f32, tag="c_out")
        nc.scalar.activation(c_out[:], pc, AF.Identity,
                             bias=vc(13), scale=1.0)


        # three-stage software pipeline:
        #   iter g emits L1(g), finishes L2(g-1), finishes L3(g-2)
        for g in range(1, NGRP + 2):
            if g < NGRP:
                if g + 2 < NGRP:
                    dma_in(g + 2)
                ps1, st1 = mm_layer("w_s1", x_tiles[g], "st1")
                kn1 = combine_stats(st1, GRP_S, vc(0), vc(1), vc(2), "l1")
                st_l1[g] = (ps1, kn1)
            h = g - 1
            if 0 <= h < NGRP:
                ps1h, (kk1, nkb1) = st_l1.pop(h)
                t1 = norm_layer(ps1h, kk1, nkb1, t1pool, "t1")
                ps2, st2 = mm_layer("w_s2", t1, "st2")
                kn2 = combine_stats(st2, GRP_S, vc(3), vc(4), vc(5), "l2")
                st_l2[h] = (ps2, kn2)
            h2 = g - 2
            if 0 <= h2 < NGRP:
                ps2h, (kk2, nkb2) = st_l2.pop(h2)
                t2 = norm_layer(ps2h, kk2, nkb2, t2pool, "t2")
                l3_gate_out(t2, h2)

    nc.compile()
    return nc


def _get_nc(with_be):
    key = ("nc", with_be)
    if key not in _CACHE:
        _CACHE[key] = _build_nc(with_be)
    return _CACHE[key]


def _prep_shard(s, k):
    """s [B, S, D] -> shard [D, S_LOC, B] for core k (blocked transpose)."""
    shard = np.empty((D, S_LOC, B), np.float32)
    base = k * S_LOC
    for i in range(0, S_LOC, 8):
        blk = s[:, base + i:base + i + 8, :]         # [B, 8, D]
        shard[:, i:i + 8, :] = blk.transpose(2, 1, 0)
    return shard


def kernel(**inputs):
    global LAST_RESULTS
    s = np.ascontiguousarray(np.asarray(inputs["s"], dtype=np.float32))
    c = np.asarray(inputs["c"], dtype=np.float32)

    def col(name):
        return np.asarray(inputs[name], dtype=np.float32).reshape(D, 1)

    g1, be1 = col("s_g1"), col("s_be1")
    g2, be2 = col("s_g2"), col("s_be2")
    cg1, cbe1 = col("c_g1"), col("c_be1")
    cg2, cbe2 = col("c_g2"), col("c_be2")
    with_be = bool(np.any(be1) or np.any(be2) or np.any(cbe1) or np.any(cbe2))
    vecs = np.concatenate([
        1.0 / (B * g1 * g1), EPS / (g1 * g1), be1,
        1.0 / (B * g2 * g2), EPS / (g2 * g2), be2,
        col("s_b3"),
        1.0 / (B * cg1 * cg1), EPS / (cg1 * cg1), cbe1,
        1.0 / (B * cg2 * cg2), EPS / (cg2 * cg2), cbe2,
        col("c_b3"),
        np.full((D, 1), 64.0, np.float32),
        np.full((D, 1), -0.5, np.float32),
    ], axis=1).astype(np.float32)

    cpack = np.concatenate(
        [np.asarray(inputs[n], np.float32).T
         for n in ["s_w1", "s_w2", "s_w3", "c_w1", "c_w2", "c_w3"]]
        + [c.T, vecs], axis=1)
    consts = dict(cpack=np.ascontiguousarray(cpack))

    in_maps = []
    for k in range(NCORES):
        m = dict(consts)
        m["xT"] = _prep_shard(s, k).reshape(D, N_LOC)
        in_maps.append(m)

    nc = _get_nc(with_be)
    res = run_bass_kernel_spmd(nc, in_maps, core_ids=list(range(NCORES)))
    LAST_RESULTS = res

    s_out = np.empty((B, S, D), np.float32)
    agg_parts = []
    for k in range(NCORES):
        o = res.results[k]["outT"].reshape(D, S_LOC, B)
        base = k * S_LOC
        for i in range(0, S_LOC, 8):
            s_out[:, base + i:base + i + 8, :] = \
                o[:, i:i + 8, :].transpose(2, 1, 0)
        agg_parts.append(o.max(axis=1))            # [D, B]
    agg = np.maximum.reduce(agg_parts).T           # [B, D]
    return s_out, np.ascontiguousarray(agg)


# revision 28
# speedup vs baseline: 1.0368x; 1.0368x over previous
"""Trainium2 Bass kernel for nn_CGBlock (gnn_message_passing).

Computation (B=256, S=512, D=128):
  c_out = c_mlp(c)                  # [B,D] MLP with BatchNorm over batch
  s_h   = s_mlp(s)                  # [B,S,D] MLP, BN stats over batch axis 0
                                    #   (independent per (seq, feature))
  s_out = s_h * c_out[:, None, :]
  agg   = max(s_out, axis=1)
  returns (s_out, agg)

Sharding: the s-MLP BatchNorm is independent per sequence position, so we
shard the SEQUENCE axis across the 8 cores (64 positions each). Every core
holds the full batch for its slice, so BN needs no cross-device collectives.
The tiny c-MLP is computed redundantly on every core.

On-device layout is feature-major ([D, seq, batch]) so the PE array can
contract over D directly; the host transposes in/out (layout prep only).

BN+ReLU is fused into one ACT instruction per sequence position:
  t = relu(kk*h + nkb),  kk = g/std,  nkb = be - mean*kk
(biases b1/b2 cancel inside BN; relu commutes with kk > 0). The stats
combine is shortened by folding per-feature scales into the ACT sqrt:
  1/kk = sqrt( (256*var) * 1/(256 g^2) + eps/g^2 )
with 1/(256 g^2) and eps/g^2 precomputed on the host.
"""
import sys
import os

for _p in ('/opt/trn_rl_repo', '/root/.axon_site/_ro/trn_rl_repo'):
    if os.path.isdir(_p) and _p not in sys.path:
        sys.path.insert(0, _p)

import numpy as np
from contextlib import ExitStack

import concourse.bass as bass
import concourse.bacc as bacc
import concourse.tile as tile
import concourse.mybir as mybir
from concourse.bass_utils import run_bass_kernel_spmd

f32 = mybir.dt.float32
f32r = mybir.dt.float32r
AF = mybir.ActivationFunctionType
ALU = mybir.AluOpType

B, S, D = 256, 512, 128
NCORES = 8
S_LOC = S // NCORES          # 64 sequence positions per core
N_LOC = S_LOC * B            # 16384 columns per core
GRP_S = 8                    # sequence positions per pipeline group
NGRP = S_LOC // GRP_S        # 8 groups
GCOLS = GRP_S * B            # 2048 columns per group
NCHUNK = GCOLS // 512        # 512-col psum chunks per group (4)
EPS = 1e-5

GATE_DVE_MOD = (0, 1, 2)     # 512-col chunks (mod 4) gated on DVE direct path

LAST_RESULTS = None
_CACHE = {}

# vecs columns (packed per-partition vectors, host-precomputed):
# 0: s1_scale = 1/(256*g1^2)    1: s1_bias = eps/g1^2   2: s_be1
# 3: s2_scale                   4: s2_bias              5: s_be2
# 6: s_b3
# 7: c1_scale                   8: c1_bias              9: c_be1
# 10: c2_scale                 11: c2_bias             12: c_be2
# 13: c_b3   14: 64.0   15: -0.5
NVEC = 16


def _build_nc(with_be):
    nc = bacc.Bacc("TRN2", target_bir_lowering=False, debug=False)

    xT = nc.dram_tensor("xT", [D, N_LOC], f32r, kind="ExternalInput").ap()
    # packed consts: 6 weights [D,D] + cT [D,B] + vecs [D,NVEC]
    CPACK = 6 * D + B + NVEC
    cpack = nc.dram_tensor("cpack", [D, CPACK], f32r,
                           kind="ExternalInput").ap()

    outT = nc.dram_tensor("outT", [D, N_LOC], f32, kind="ExternalOutput").ap()

    with tile.TileContext(nc) as tc, ExitStack() as ctx:
        cpool = ctx.enter_context(tc.tile_pool(name="consts", bufs=1))
        xpool = ctx.enter_context(tc.tile_pool(name="xin", bufs=4))
        t1pool = ctx.enter_context(tc.tile_pool(name="t1", bufs=3))
        t2pool = ctx.enter_context(tc.tile_pool(name="t2", bufs=3))
        sopool = ctx.enter_context(tc.tile_pool(name="sout", bufs=3))
        stpool = ctx.enter_context(tc.tile_pool(name="stats", bufs=4))
        smpool = ctx.enter_context(tc.tile_pool(name="smalls", bufs=4))
        cmpool = ctx.enter_context(tc.tile_pool(name="cmlp", bufs=2))
        pspool = ctx.enter_context(tc.tile_pool(name="ps", bufs=8, space="PSUM"))

        # ---- load constants (single DMA) ----
        wnames = ["w_s1", "w_s2", "w_s3", "w_c1", "w_c2", "w_c3"]
        cpk = cpool.tile([D, CPACK], f32r, tag="cpack")
        nc.sync.dma_start(cpk[:], cpack)
        wts = {nm: cpk[:, i * D:(i + 1) * D] for i, nm in enumerate(wnames)}
        ct = cpk[:, 6 * D:6 * D + B]
        vt = cpk[:, 6 * D + B:].bitcast(f32)

        def vc(i):
            return vt[:, i:i + 1]

        s_b3 = vc(6)

        # ---- stats combine: even/odd bn_stats -> (kk, nkb) [128, n] ----
        # kk = g/std ; nkb = be - mean*kk
        def combine_stats(st, n, sc_ap, sb_ap, be_ap, tag):
            me = st[:, :, 1]; mo = st[:, :, 4]
            cve = st[:, :, 2]; cvo = st[:, :, 5]
            Ssum = smpool.tile([D, n], f32, tag=f"S_{tag}")
            nc.gpsimd.tensor_tensor(Ssum[:], me, mo, op=ALU.add)
            Dd = smpool.tile([D, n], f32, tag=f"D_{tag}")
            nc.vector.tensor_tensor(Dd[:], me, mo, op=ALU.subtract)
            D2 = smpool.tile([D, n], f32, tag=f"D2_{tag}")
            nc.vector.tensor_tensor(D2[:], Dd[:], Dd[:], op=ALU.mult)
            Q = smpool.tile([D, n], f32, tag=f"Q_{tag}")
            nc.vector.tensor_tensor(Q[:], cve, cvo, op=ALU.add)
            # w = 256*var = D2*64 + Q
            w = smpool.tile([D, n], f32, tag=f"w_{tag}")
            nc.vector.scalar_tensor_tensor(w[:], D2[:], 64.0, Q[:],
                                           op0=ALU.mult, op1=ALU.add)
            # kk = g/std = rsqrt(w*sc + sb)  (one ACT op; arg is positive)
            kk = smpool.tile([D, n], f32, tag=f"kk_{tag}")
            nc.scalar.activation(kk[:], w[:], AF.Abs_reciprocal_sqrt,
                                 bias=sb_ap, scale=sc_ap)
            nkb = smpool.tile([D, n], f32, tag=f"nkb_{tag}")
            nc.vector.scalar_tensor_tensor(nkb[:], Ssum[:], -0.5, kk[:],
                                           op0=ALU.mult, op1=ALU.mult)
            if with_be:
                nkb2 = smpool.tile([D, n], f32, tag=f"nkb2_{tag}")
                nc.vector.tensor_scalar(nkb2[:], nkb[:], be_ap, None,
                                        op0=ALU.add)
                nkb = nkb2
            return kk, nkb

        # ---- s-MLP: software-pipelined over groups ----
        x_tiles = [None] * NGRP

        def dma_in(g):
            xg = xpool.tile([D, GCOLS], f32r, tag="xg")
            nc.gpsimd.dma_start(xg[:], xT[:, g * GCOLS:(g + 1) * GCOLS])
            x_tiles[g] = xg

        def mm_layer(wname, src_tile, sttag):
            """4 matmuls [D,512] + 8 per-s bn_stats."""
            ps = []
            st = stpool.tile([D, GRP_S, 6], f32, tag=sttag)
            for cchunk in range(NCHUNK):
                pt = pspool.tile([D, 512], f32, tag="pt")
                nc.tensor.matmul(pt[:], wts[wname],
                                 src_tile[:, cchunk * 512:(cchunk + 1) * 512],
                                 start=True, stop=True)
                for half in range(2):
                    si = cchunk * 2 + half
                    nc.vector.bn_stats(st[:, si, :],
                                       pt[:, half * B:(half + 1) * B])
                ps.append(pt)
            return ps, st

        def norm_layer(ps, kk, nkb, dpool, dtag):
            dst_t = dpool.tile([D, GCOLS], f32r, tag=dtag)
            for si in range(GRP_S):
                pt = ps[si // 2]
                src = pt[:, (si % 2) * B:(si % 2 + 1) * B]
                nc.scalar.activation(dst_t[:, si * B:(si + 1) * B], src,
                                     AF.Relu, bias=nkb[:, si:si + 1],
                                     scale=kk[:, si:si + 1])
            return dst_t

        def l3_gate_out(t2, g):
            so = sopool.tile([D, GCOLS], f32, tag="so")
            for spair in range(NCHUNK):
                pt = pspool.tile([D, 512], f32, tag="pt")
                nc.tensor.matmul(pt[:], wts["w_s3"],
                                 t2[:, spair * 512:(spair + 1) * 512],
                                 start=True, stop=True)
                dstv = so[:, spair * 512:(spair + 1) * 512]
                if spair % 4 in GATE_DVE_MOD:
                    nc.vector.scalar_tensor_tensor(
                        dstv.rearrange("p (s b) -> p s b", b=B),
                        pt[:].rearrange("p (s b) -> p s b", b=B),
                        s_b3,
                        c_out[:].unsqueeze(1).broadcast_to([D, 2, B]),
                        op0=ALU.add, op1=ALU.mult)
                else:
                    u = smpool.tile([D, 512], f32, tag="u")
                    nc.scalar.activation(u[:], pt[:], AF.Identity,
                                         bias=s_b3, scale=1.0)
                    nc.gpsimd.tensor_tensor(
                        dstv.rearrange("p (s b) -> p s b", b=B),
                        u[:].rearrange("p (s b) -> p s b", b=B),
                        c_out[:].unsqueeze(1).broadcast_to([D, 2, B]),
                        op=ALU.mult)
                nc.sync.dma_start(
                    outT[:, g * GCOLS + spair * 512:
                         g * GCOLS + (spair + 1) * 512], dstv)

        # prologue
        dma_in(0)
        dma_in(1)
        dma_in(2)
        st_l1 = {}
        st_l2 = {}
        ps1, st1 = mm_layer("w_s1", x_tiles[0], "st1")
        kn1 = combine_stats(st1, GRP_S, vc(0), vc(1), vc(2), "l1")
        st_l1[0] = (ps1, kn1)

        # ---- c-MLP (tiny, replicated on every core) ----
        cc = ct
        for li, (wn, sc_i, sb_i, be_i) in enumerate(
                [("w_c1", 7, 8, 9), ("w_c2", 10, 11, 12)]):
            pcf = pspool.tile([D, 512], f32, tag="pt")
            pc = pcf[:, 0:B]
            nc.tensor.matmul(pc, wts[wn], cc[:], start=True, stop=True)
            st = stpool.tile([D, 1, 6], f32, tag=f"stc{li}")
            nc.vector.bn_stats(st[:, 0, :], pc)
            kk, nkb = combine_stats(st, 1, vc(sc_i), vc(sb_i), vc(be_i),
                                    f"c{li}")
            nxt = cmpool.tile([D, B], f32r, tag=f"cact{li}")
            nc.scalar.activation(nxt[:], pc, AF.Relu,
                                 bias=nkb[:, 0:1], scale=kk[:, 0:1])
            cc = nxt
        pcf = pspool.tile([D, 512], f32, tag="pt")
        pc = pcf[:, 0:B]
        nc.tensor.matmul(pc, wts["w_c3"], cc[:], start=True, stop=True)
        c_out = cpool.tile([D, B], f32, tag="c_out")
        nc.scalar.activation(c_out[:], pc, AF.Identity,
                             bias=vc(13), scale=1.0)


        # three-stage software pipeline:
        #   iter g emits L1(g), finishes L2(g-1), finishes L3(g-2)
        for g in range(1, NGRP + 2):
            if g < NGRP:
                if g + 2 < NGRP:
                    dma_in(g + 2)
                ps1, st1 = mm_layer("w_s1", x_tiles[g], "st1")
                kn1 = combine_stats(st1, GRP_S, vc(0), vc(1), vc(2), "l1")
                st_l1[g] = (ps1, kn1)
            h = g - 1
            if 0 <= h < NGRP:
                ps1h, (kk1, nkb1) = st_l1.pop(h)
                t1 = norm_layer(ps1h, kk1, nkb1, t1pool, "t1")
                ps2, st2 = mm_layer("w_s2", t1, "st2")
                kn2 = combine_stats(st2, GRP_S, vc(3), vc(4), vc(5), "l2")
                st_l2[h] = (ps2, kn2)
            h2 = g - 2
            if 0 <= h2 < NGRP:
                ps2h, (kk2, nkb2) = st_l2.pop(h2)
                t2 = norm_layer(ps2h, kk2, nkb2, t2pool, "t2")
                l3_gate_out(t2, h2)

    nc.compile()
    return nc


def _get_nc(with_be):
    key = ("nc", with_be)
    if key not in _CACHE:
        _CACHE[key] = _build_nc(with_be)
    return _CACHE[key]


def _prep_shard(s, k):
    """s [B, S, D] -> shard [D, S_LOC, B] for core k (blocked transpose)."""
    shard = np.empty((D, S_LOC, B), np.float32)
    base = k * S_LOC
    for i in range(0, S_LOC, 8):
        blk = s[:, base + i:base + i + 8, :]         # [B, 8, D]
        shard[:, i:i + 8, :] = blk.transpose(2, 1, 0)
    return shard


def kernel(**inputs):
    global LAST_RESULTS
    s = np.ascontiguousarray(np.asarray(inputs["s"], dtype=np.float32))
    c = np.asarray(inputs["c"], dtype=np.float32)

    def col(name):
        return np.asarray(inputs[name], dtype=np.float32).reshape(D, 1)

    g1, be1 = col("s_g1"), col("s_be1")
    g2, be2 = col("s_g2"), col("s_be2")
    cg1, cbe1 = col("c_g1"), col("c_be1")
    cg2, cbe2 = col("c_g2"), col("c_be2")
    with_be = bool(np.any(be1) or np.any(be2) or np.any(cbe1) or np.any(cbe2))
    vecs = np.concatenate([
        1.0 / (B * g1 * g1), EPS / (g1 * g1), be1,
        1.0 / (B * g2 * g2), EPS / (g2 * g2), be2,
        col("s_b3"),
        1.0 / (B * cg1 * cg1), EPS / (cg1 * cg1), cbe1,
        1.0 / (B * cg2 * cg2), EPS / (cg2 * cg2), cbe2,
        col("c_b3"),
        np.full((D, 1), 64.0, np.float32),
        np.full((D, 1), -0.5, np.float32),
    ], axis=1).astype(np.float32)

    cpack = np.concatenate(
        [np.asarray(inputs[n], np.float32).T
         for n in ["s_w1", "s_w2", "s_w3", "c_w1", "c_w2", "c_w3"]]
        + [c.T, vecs], axis=1)
    consts = dict(cpack=np.ascontiguousarray(cpack))

    in_maps = []
    for k in range(NCORES):
        m = dict(consts)
        m["xT"] = _prep_shard(s, k).reshape(D, N_LOC)
        in_maps.append(m)

    nc = _get_nc(with_be)
    res = run_bass_kernel_spmd(nc, in_maps, core_ids=list(range(NCORES)))
    LAST_RESULTS = res

    s_out = np.empty((B, S, D), np.float32)
    agg_parts = []
    for k in range(NCORES):
        o = res.results[k]["outT"].reshape(D, S_LOC, B)
        base = k * S_LOC
        for i in range(0, S_LOC, 8):
            s_out[:, base + i:base + i + 8, :] = \
                o[:, i:i + 8, :].transpose(2, 1, 0)
        agg_parts.append(o.max(axis=1))            # [D, B]
    agg = np.maximum.reduce(agg_parts).T           # [B, D]
    return s_out, np.ascontiguousarray(agg)
